# revision 7
# baseline (speedup 1.0000x reference)
"""Causal self-attention (ALiBi + QK-RMSNorm + subln) on 8 TRN2 NeuronCores.

Sharding: 8 cores = 2 batches x 4 head-groups (4 heads / 128 head-dim each).
Per core: QKV projection for its 512 features, attention for its 4 heads,
and a partial output projection (row slice of Wproj); host sums the 4
partials per batch.

All matmuls run as float32r (full-rate fp32 streaming with ~11-bit input
rounding, fp32 accumulation).

v2 notes:
- One ACT table set for the whole program (natural_log_exp_and_others,
  pre-placed load): every rsqrt is computed as exp(-0.5*ln(x)), so the
  per-chunk sqrt<->exp table thrash (and the HAM re-throttle it caused)
  is gone.
- k-RMS (rk) is folded into k-hat during phase A, so every softmax exp
  runs with scale=1.0 and a per-head constant bias column -> exps for
  the shallow-slope heads span two j-tiles ([128,1024] PSUM, 2 banks).
- The ALiBi block factor for the single-exp heads is pinned to global
  j (baked into the resident v tiles at drain time); the per-i
  compensation becomes chunk-dependent and is DMA'd per chunk.
- PE warm-up matmuls at kernel start keep the HAM clock-gate open
  through the initial DMA ramp.
"""
import math

import numpy as np

import concourse.bacc as bacc
import concourse.bass as bass
import concourse.mybir as mybir
from concourse.tile import TileContext

F32 = mybir.dt.float32
F32R = mybir.dt.float32r
AF = mybir.ActivationFunctionType

B, T, C = 2, 2048, 2048
TC_G = 4
H, D = 16, 128
HG = 4          # head groups = cores per batch
HPG = 4         # heads per group
F = HPG * D     # 512 per-core qkv features
EPS = 1e-5
NEG = -1.0e30
PIN = 64        # diagonal pinning offset for ALiBi exp factorization
K_PIN = 1024    # token pin for the jt-global v factor (single-exp heads)
LN_G = 5.545177444479562  # ln(256): global exp gain so unnormalized rows
                          # stay well above the subln eps floor
ACT_SET_LN_EXP = 6        # act_info.json index of natural_log_exp_and_others
NWARM = 64                # PE warm-up matmuls covering the initial DMA ramp


def _alibi_slopes(n_heads):
    def pow2(n):
        start = 2 ** (-(2 ** (-(math.log2(n) - 3))))
        return [start * start**i for i in range(n)]

    if math.log2(n_heads).is_integer():
        return pow2(n_heads)
    c = 2 ** math.floor(math.log2(n_heads))
    s = pow2(c)
    extra = _alibi_slopes(2 * c)
    return s + extra[0::2][: n_heads - c]


def _build():
    nc = bacc.Bacc("TRN2", target_bir_lowering=False)

    TC = T // 512  # 4 t-chunks

    xt = nc.dram_tensor("xt", [C, T], F32R, kind="ExternalInput")
    wq = nc.dram_tensor("wq", [C, F], F32R, kind="ExternalInput")
    wk = nc.dram_tensor("wk", [C, F], F32R, kind="ExternalInput")
    wv = nc.dram_tensor("wv", [C, F], F32R, kind="ExternalInput")
    wp = nc.dram_tensor("wp", [F, C], F32R, kind="ExternalInput")
    bias_tab = nc.dram_tensor("bias_tab", [128, HPG * 16], F32, kind="ExternalInput")
    cmask = nc.dram_tensor("cmask", [128, 128], F32, kind="ExternalInput")
    ones_c = nc.dram_tensor("ones_c", [128, 1], F32R, kind="ExternalInput")
    ones_r = nc.dram_tensor("ones_r", [1, 128], F32R, kind="ExternalInput")
    vfac = nc.dram_tensor("vfac", [128, 2 * 16], F32, kind="ExternalInput")
    comp = nc.dram_tensor("comp", [128, TC * HPG * 512], F32, kind="ExternalInput")
    out = nc.dram_tensor("out", [T, C], F32, kind="ExternalOutput")

    # per-chunk q scratch: attention chunk ic only waits on chunk ic's
    # q stores, so phase B overlaps the q pass
    qt_s = [
        nc.dram_tensor(f"qt_s{ic}", [F, 512], F32R)
        for ic in range(TC)
    ]
    # per-head k scratch so phase B's reload of head h only waits on the
    # stores of head h (overlaps the tail of phase A)
    kt_s = [
        nc.dram_tensor(f"kt_s{ft}", [128, T], F32R)
        for ft in range(HPG)
    ]

    with nc.allow_low_precision(reason="f32r rounding of matmul operands is intentional"), TileContext(nc) as tc:
        # Single table set for the whole program (exp + ln + copy); the
        # fixpoint load-insertion pass sees it covers every activation
        # and inserts nothing else.
        nc.scalar.add_instruction(mybir.InstLoadActFuncSet(
            name=nc.get_next_instruction_name(),
            act_func_set_id=ACT_SET_LN_EXP,
        ))
        with (
            tc.tile_pool(name="consts", bufs=1) as consts,
            tc.tile_pool(name="psum", bufs=1, space="PSUM") as psum,
        ):
            bias_t = consts.tile([128, HPG * 16], F32, tag="bias_t")
            nc.sync.dma_start(out=bias_t, in_=bias_tab[:, :])
            mask_t = consts.tile([128, 128], F32, tag="mask_t")
            nc.sync.dma_start(out=mask_t, in_=cmask[:, :])
            onesc_t = consts.tile([128, 1], F32R, tag="onesc_t")
            nc.sync.dma_start(out=onesc_t, in_=ones_c[:, :])
            onesr_t = consts.tile([1, 128], F32R, tag="onesr_t")
            nc.sync.dma_start(out=onesr_t, in_=ones_r[:, :])
            vfac_t = consts.tile([128, 2 * 16], F32, tag="vfac_t")
            nc.sync.dma_start(out=vfac_t, in_=vfac[:, :])
            eps_c = consts.tile([128, 1], F32, tag="eps_c")
            nc.vector.memset(eps_c, EPS)
            eps_c1 = consts.tile([1, 1], F32, tag="eps_c1")
            nc.vector.memset(eps_c1, EPS)
            eps128_r = consts.tile([1, 1], F32, tag="eps128_r")
            nc.vector.memset(eps128_r, 128.0 * EPS)
            zeros_r = consts.tile([128, 512], F32R, tag="zeros_r")
            nc.vector.memset(zeros_r.bitcast(F32), 0.0)
            warm_sink = consts.tile([128, 1], F32, tag="warm_sink")

            # PE warm-up: back-to-back matmuls on the zero tile so the
            # HAM clock-gate opens during the initial DMA ramp and the
            # first real matmul starts at 2.4 GHz.
            for w in range(NWARM):
                wps = psum.tile([128, 512], F32, tag="big_ps", bufs=2)
                nc.tensor.matmul(
                    wps, zeros_r[:, 0:128], zeros_r, start=True, stop=True
                )
                if w == NWARM - 1:
                    nc.vector.tensor_copy(warm_sink, wps[:, 0:1])

            # v stays resident in SBUF (f32r), written directly from the
            # projection PSUM in phase A -- no DRAM round trip for v.
            # Heads 2/3 (shallow slopes) are pre-scaled by the jt-global
            # ALiBi block factor at drain time.
            vh_all = []
            for h in range(HPG):
                vht = consts.tile([128, 16, 128], F32R, tag=f"v_h{h}")
                vh_all.append(vht)

            xt_r = xt.rearrange("(ct p) t -> p ct t", p=128)

            # ------- Phase A pass 1: k + v (x streamed in per-ct) ---------
            # k/v finish before the q pass even starts, so phase B's k
            # reloads and first attention chunks overlap the whole q pass.
            with tc.tile_pool(name="xpool", bufs=2) as xpool:
              with (
                tc.tile_pool(name="wkv", bufs=1) as wkv,
                tc.tile_pool(name="stg", bufs=2) as stg,
              ):
                wk_t = wkv.tile([128, 16, F], F32R, tag="wk_t")
                wv_t = wkv.tile([128, 16, F], F32R, tag="wv_t")
                x_first = xpool.tile([128, 16, 512], F32R, tag="x_t")
                for ct in range(16):
                    nc.sync.dma_start(
                        out=wk_t[:, ct, :], in_=wk[ct * 128:(ct + 1) * 128, :]
                    )
                    nc.sync.dma_start(
                        out=x_first[:, ct, :], in_=xt_r[:, ct, 0:512]
                    )
                for ct in range(16):
                    nc.sync.dma_start(
                        out=wv_t[:, ct, :], in_=wv[ct * 128:(ct + 1) * 128, :]
                    )

                for tch in range(TC):
                    if tch == 0:
                        x_t = x_first
                    else:
                        x_t = xpool.tile([128, 16, 512], F32R, tag="x_t")
                        for ct in range(16):
                            nc.sync.dma_start(
                                out=x_t[:, ct, :],
                                in_=xt_r[:, ct, tch * 512:(tch + 1) * 512],
                            )

                    # ---- k: project, rms-normalize per token (rk folded
                    #      in so phase B exps run at scale=1), store
                    for ft in range(4):
                        ps = psum.tile([128, 512], F32, tag="big_ps", bufs=2)
                        for ct in range(16):
                            nc.tensor.matmul(
                                ps,
                                wk_t[:, ct, ft * 128:(ft + 1) * 128],
                                x_t[:, ct, :],
                                start=(ct == 0),
                                stop=(ct == 15),
                            )
                        st = stg.tile([128, 512], F32R, tag="st", bufs=3)
                        nc.scalar.copy(st, ps)
                        ksq = stg.tile([128, 512], F32R, tag="sq")
                        nc.vector.tensor_mul(ksq, st.bitcast(F32), st.bitcast(F32))
                        # tags shared with phase B (8-bank PSUM budget):
                        # row sum sits in partition 0 of a proj_ps tile,
                        # the broadcast in the lower half of an s_ps tile
                        ps_row_t = psum.tile([128, 512], F32, tag="proj_ps", bufs=2)
                        ps_row = ps_row_t[0:1, :]
                        nc.tensor.matmul(ps_row, onesc_t, ksq, start=True, stop=True)
                        rln = stg.tile([1, 512], F32, tag="rln", bufs=2)
                        nc.scalar.activation(
                            rln, ps_row, AF.Ln, scale=1.0 / 128.0, bias=eps_c1
                        )
                        rk_f = stg.tile([1, 512], F32, tag="rk_f", bufs=2)
                        nc.scalar.activation(rk_f, rln, AF.Exp, scale=-0.5)
                        rk_row = stg.tile([1, 512], F32R, tag="rk_row", bufs=2)
                        nc.vector.tensor_copy(rk_row, rk_f)
                        ps_b_t = psum.tile([128, 1024], F32, tag="s_ps", bufs=2)
                        ps_b = ps_b_t[:, 0:512]
                        nc.tensor.matmul(ps_b, onesr_t, rk_row, start=True, stop=True)
                        khat = stg.tile([128, 512], F32R, tag="khat_a", bufs=2)
                        nc.vector.tensor_mul(khat, st.bitcast(F32), ps_b)
                        nc.sync.dma_start(
                            out=kt_s[ft][:, tch * 512:(tch + 1) * 512],
                            in_=khat,
                        )

                    # ---- v: copy per-head slices straight into the
                    #      resident tiles (ACT/DVE split); heads 2/3 get
                    #      the jt-global ALiBi block factor baked in
                    for ts4 in range(4):
                        ps = psum.tile([128, 512], F32, tag="big_ps", bufs=2)
                        for ct in range(16):
                            nc.tensor.matmul(
                                ps,
                                x_t[:, ct, ts4 * 128:(ts4 + 1) * 128],
                                wv_t[:, ct, :],
                                start=(ct == 0),
                                stop=(ct == 15),
                            )
                        jt = tch * 4 + ts4
                        for h in range(HPG):
                            dst = vh_all[h][:, jt, :]
                            src = ps[:, h * 128:(h + 1) * 128]
                            if h < 2:
                                if h == 0:
                                    nc.scalar.copy(dst, src)
                                else:
                                    nc.vector.tensor_copy(dst, src)
                            else:
                                fcol = vfac_t[:, (h - 2) * 16 + jt:
                                              (h - 2) * 16 + jt + 1]
                                if h == 2:
                                    nc.scalar.mul(dst, src, fcol)
                                else:
                                    nc.vector.tensor_scalar_mul(
                                        dst, src, scalar1=fcol
                                    )

              # ------- Phase A pass 2: q (x streamed in again) ------------
              with (
                tc.tile_pool(name="wqp", bufs=1) as wqp,
                tc.tile_pool(name="stg2", bufs=2) as stg2,
              ):
                wq_t = wqp.tile([128, 16, F], F32R, tag="wq_t")
                for ct in range(16):
                    nc.sync.dma_start(
                        out=wq_t[:, ct, :], in_=wq[ct * 128:(ct + 1) * 128, :]
                    )

                for tch in range(TC):
                    x_t = xpool.tile([128, 16, 512], F32R, tag="x_t")
                    for ct in range(16):
                        nc.sync.dma_start(
                            out=x_t[:, ct, :],
                            in_=xt_r[:, ct, tch * 512:(tch + 1) * 512],
                        )

                    # ---- q: project, rms-normalize (1/sqrt(D) folded), store
                    for ft in range(4):
                        ps = psum.tile([128, 512], F32, tag="big_ps", bufs=2)
                        for ct in range(16):
                            nc.tensor.matmul(
                                ps,
                                wq_t[:, ct, ft * 128:(ft + 1) * 128],
                                x_t[:, ct, :],
                                start=(ct == 0),
                                stop=(ct == 15),
                            )
                        st = stg2.tile([128, 512], F32R, tag="st", bufs=3)
                        nc.scalar.copy(st, ps)
                        qsq = stg2.tile([128, 512], F32R, tag="sq")
                        nc.vector.tensor_mul(qsq, st.bitcast(F32), st.bitcast(F32))
                        ps_row_t = psum.tile([128, 512], F32, tag="proj_ps", bufs=2)
                        ps_row = ps_row_t[0:1, :]
                        nc.tensor.matmul(ps_row, onesc_t, qsq, start=True, stop=True)
                        rln2 = stg2.tile([1, 512], F32, tag="rln2", bufs=2)
                        nc.scalar.activation(
                            rln2, ps_row, AF.Ln, scale=1.0, bias=eps128_r
                        )
                        rq_f = stg2.tile([1, 512], F32, tag="rq_f", bufs=2)
                        nc.scalar.activation(rq_f, rln2, AF.Exp, scale=-0.5)
                        rq_row = stg2.tile([1, 512], F32R, tag="rq_row", bufs=2)
                        nc.vector.tensor_copy(rq_row, rq_f)
                        ps_b_t = psum.tile([128, 1024], F32, tag="s_ps", bufs=2)
                        ps_b = ps_b_t[:, 0:512]
                        nc.tensor.matmul(ps_b, onesr_t, rq_row, start=True, stop=True)
                        qhat = stg2.tile([128, 512], F32R, tag="qhat", bufs=3)
                        nc.vector.tensor_mul(qhat, st.bitcast(F32), ps_b)
                        nc.sync.dma_start(
                            out=qt_s[tch][ft * 128:(ft + 1) * 128, :],
                            in_=qhat,
                        )

            # ---------------- Phase B: attention per head -----------------
            with (
                tc.tile_pool(name="head", bufs=1) as head,
                tc.tile_pool(name="ppool", bufs=2) as ppool,
                tc.tile_pool(name="yfin_pool", bufs=1) as yfin_pool,
                tc.tile_pool(name="small", bufs=2) as small,
                tc.tile_pool(name="opool", bufs=2) as opool,
            ):
                yfin = []
                for h in range(HPG):
                    yf = yfin_pool.tile([128, T], F32R, tag=f"yfin{h}")
                    yfin.append(yf)

                # k reloads (per-head scratch: each overlaps phase A's tail)
                khat_by_h = {}
                v_by_h = dict(enumerate(vh_all))
                for h in range(HPG):
                    kh = head.tile([128, T], F32R, tag=f"khat{h}", bufs=1)
                    nc.sync.dma_start(out=kh, in_=kt_s[h][:, :])
                    khat_by_h[h] = kh
                wp_t = head.tile([128, HPG, C], F32R, tag="wp_t", bufs=1)
                nc.sync.dma_start(
                    out=wp_t, in_=wp.rearrange("(ht p) c -> p ht c", p=128)
                )

                comp_by_ic = {}
                ysr_by_ic = {}

                def emit_comp_dma(ic):
                    ct = small.tile([128, HPG * 512], F32, tag="comp_t",
                                    bufs=2)
                    nc.sync.dma_start(
                        out=ct,
                        in_=comp[:, ic * HPG * 512:(ic + 1) * HPG * 512],
                    )
                    comp_by_ic[ic] = ct

                def finish_chunk(st):
                    # y_unnorm times the per-(chunk, head) compensation
                    # vector (removes the exp-pinning factors). The
                    # softmax denominator is skipped entirely: subln
                    # rmsnorm is invariant to any positive per-row scale,
                    # and the G gain keeps rows far above the eps floor.
                    h, ic, y_ps = st["h"], st["ic"], st["y_ps"]
                    yslice = yfin[h][:, ic * 512:(ic + 1) * 512]
                    nc.vector.tensor_mul(
                        yslice, y_ps,
                        comp_by_ic[ic][:, h * 512:(h + 1) * 512],
                    )
                    ysq = small.tile([128, 512], F32R, tag="ysq")
                    nc.vector.tensor_mul(
                        ysq, yslice.bitcast(F32), yslice.bitcast(F32)
                    )
                    # row-sum lands in partition 0 of the (already
                    # drained) y_ps bank -- no extra PSUM bank needed
                    nc.tensor.matmul(
                        y_ps[0:1, :], onesc_t, ysq, start=True, stop=True
                    )
                    if ic not in ysr_by_ic:
                        ysr = small.tile(
                            [1, HPG * 512], F32, tag="ysr", bufs=2,
                            name=f"ysr{ic}",
                        )
                        ysr_by_ic[ic] = ysr
                    nc.vector.tensor_copy(
                        ysr_by_ic[ic][:, h * 512:(h + 1) * 512], y_ps[0:1, :]
                    )

                def emit_head_chunk(h, ic):
                    qhat = small.tile([128, 512], F32R, tag="qhat", bufs=2)
                    nc.sync.dma_start(
                        out=qhat,
                        in_=qt_s[ic][h * 128:(h + 1) * 128, :],
                    )
                    njt = 4 * ic + 4
                    single_exp = h >= 2
                    y_ps = psum.tile([128, 512], F32, tag="big_ps", bufs=2)
                    # slot-0 slopes >= 0.25 (drop diff>=2), slot-1 >= 0.0625
                    # (drop diff>=4); dropped weights are < e^-24 relative.
                    dmax = {0: 1, 1: 3}.get(h, 99)
                    jt0 = max(0, 4 * ic - dmax) if h < 2 else 0
                    khat = khat_by_h[h]
                    v_h = v_by_h[h]

                    for pb in range(jt0 - (jt0 % 2), njt, 2):
                        units = [u for u in (pb, pb + 1) if u >= jt0]
                        s_ps = psum.tile([128, 1024], F32, tag="s_ps", bufs=2)
                        for u in units:
                            nc.tensor.matmul(
                                s_ps[:, (u - pb) * 512:(u - pb + 1) * 512],
                                khat[:, u * 128:(u + 1) * 128],
                                qhat, start=True, stop=True,
                            )
                        pt = ppool.tile([128, 1024], F32R, tag="pt")
                        if single_exp:
                            for u in units:
                                off = (u - pb) * 512
                                i_lo = max(0, u - 4 * ic)
                                if i_lo > 0:
                                    nc.vector.tensor_scalar_add(
                                        s_ps[:, off:off + i_lo * 128],
                                        s_ps[:, off:off + i_lo * 128],
                                        scalar1=NEG,
                                    )
                                if u >= 4 * ic:
                                    isub = u - 4 * ic
                                    blk = s_ps[:, off + isub * 128:
                                               off + (isub + 1) * 128]
                                    nc.vector.tensor_add(blk, blk, mask_t)
                            # one exp across both j-tiles (2 PSUM banks)
                            nc.scalar.activation(
                                pt, s_ps, AF.Exp, scale=1.0,
                                bias=bias_t[:, h * 16:h * 16 + 1],
                            )
                            for u in units:
                                nc.tensor.matmul(
                                    y_ps, v_h[:, u, :],
                                    pt[:, (u - pb) * 512:(u - pb + 1) * 512],
                                    start=(u == jt0), stop=(u == njt - 1),
                                    skip_group_check=True,
                                )
                        else:
                            # h<2: ALiBi window -- only sub-blocks within
                            # dmax of the diagonal are exped; the rest of
                            # pt is zero-filled.
                            for u in units:
                                off = (u - pb) * 512
                                i_lo = max(0, u - 4 * ic)
                                i_hi = min(3, u + dmax - 4 * ic)
                                if i_lo > 0:
                                    nc.vector.tensor_copy(
                                        pt[:, off:off + i_lo * 128],
                                        zeros_r[:, 0:i_lo * 128],
                                    )
                                if i_hi < 3:
                                    nc.vector.tensor_copy(
                                        pt[:, off + (i_hi + 1) * 128:off + 512],
                                        zeros_r[:, 0:(3 - i_hi) * 128],
                                    )
                                for isub in range(i_lo, i_hi + 1):
                                    diff = 4 * ic + isub - u
                                    blk = s_ps[:, off + isub * 128:
                                               off + (isub + 1) * 128]
                                    if diff == 0:
                                        nc.vector.tensor_add(blk, blk, mask_t)
                                    nc.scalar.activation(
                                        pt[:, off + isub * 128:
                                           off + (isub + 1) * 128],
                                        blk, AF.Exp, scale=1.0,
                                        bias=bias_t[:, h * 16 + diff:
                                                    h * 16 + diff + 1],
                                    )
                                nc.tensor.matmul(
                                    y_ps, v_h[:, u, :],
                                    pt[:, off:off + 512],
                                    start=(u == jt0), stop=(u == njt - 1),
                                    skip_group_check=True,
                                )
                    return dict(h=h, ic=ic, y_ps=y_ps)

                def emit_subln_rstd(ic):
                    # subln rstd for chunk ic via exp(-0.5*ln(mean+eps)) --
                    # stays in the exp/ln table set, no table switch.
                    ysr = ysr_by_ic.pop(ic)
                    rln3 = small.tile([1, HPG * 512], F32, tag="rln3",
                                      bufs=1)
                    nc.scalar.activation(
                        rln3, ysr, AF.Ln, scale=1.0 / 128.0, bias=eps_c1
                    )
                    rstd = small.tile([1, HPG * 512], F32, tag="rstd",
                                      bufs=1)
                    nc.scalar.activation(rstd, rln3, AF.Exp, scale=-0.5)
                    for h in range(HPG):
                        ysb = small.tile([128, 512], F32, tag="ysb", bufs=2)
                        nc.gpsimd.partition_broadcast(
                            ysb, rstd[:, h * 512:(h + 1) * 512], 128
                        )
                        yslice = yfin[h][:, ic * 512:(ic + 1) * 512]
                        nc.vector.tensor_mul(
                            yslice, yslice.bitcast(F32), ysb
                        )

                def emit_proj_quad(ic, cc):
                    # output-projection tiles (tt, cc) for chunk ic, one
                    # cc column slice; interleaved per head so these PE
                    # matmuls fill the exp-bound stretches.
                    for tt in range(4 * ic, 4 * ic + 4):
                        ps = psum.tile([128, 512], F32, tag="proj_ps",
                                       bufs=2)
                        for h in range(HPG):
                            nc.tensor.matmul(
                                ps,
                                yfin[h][:, tt * 128:(tt + 1) * 128],
                                wp_t[:, h, cc * 512:(cc + 1) * 512],
                                start=(h == 0),
                                stop=(h == HPG - 1),
                            )
                        ot = opool.tile([128, 512], F32, tag="ot")
                        nc.vector.tensor_copy(ot, ps)
                        nc.sync.dma_start(
                            out=out[tt * 128:(tt + 1) * 128,
                                    cc * 512:(cc + 1) * 512],
                            in_=ot,
                        )

                pending = None
                for ic in range(TC):
                    emit_comp_dma(ic)
                    if ic >= 1:
                        emit_subln_rstd(ic - 1)
                    for h in range(HPG):
                        st = emit_head_chunk(h, ic)
                        if pending is not None:
                            finish_chunk(pending)
                        pending = st
                        if ic >= 1:
                            emit_proj_quad(ic - 1, h)
                    finish_chunk(pending)
                    pending = None
                emit_subln_rstd(TC - 1)
                for cc in range(4):
                    emit_proj_quad(TC - 1, cc)

    nc.compile()
    return nc


_NC_CACHE = None


def _get_nc():
    global _NC_CACHE
    if _NC_CACHE is None:
        _NC_CACHE = _build()
    return _NC_CACHE


def kernel_in_maps(x, Wq, Wk, Wv, Wproj, q_rms_w, k_rms_w, subln_w):
    slopes = _alibi_slopes(H)
    TC = T // 512

    x = np.asarray(x, dtype=np.float32)
    Wq = np.asarray(Wq, dtype=np.float32)
    Wk = np.asarray(Wk, dtype=np.float32)
    Wv = np.asarray(Wv, dtype=np.float32)
    Wproj = np.asarray(Wproj, dtype=np.float32)
    q_rms_w = np.asarray(q_rms_w, dtype=np.float32)
    k_rms_w = np.asarray(k_rms_w, dtype=np.float32)
    subln_w = np.asarray(subln_w, dtype=np.float32)

    wqk = np.tile((q_rms_w * k_rms_w).astype(np.float64), HPG)  # (512,)
    cmask = np.where(
        np.arange(128)[:, None] <= np.arange(128)[None, :], 0.0, NEG
    ).astype(np.float32)
    ones_c = np.ones((128, 1), np.float32)
    ones_r = np.ones((1, 128), np.float32)
    dj = np.arange(128, dtype=np.float64)

    in_maps = []
    for b in range(B):
        xt = np.ascontiguousarray(x[b].T)
        for g in range(HG):
            heads = [g + 4 * j for j in range(HPG)]  # strided: slopes shrink with j
            csel = np.concatenate(
                [np.arange(hh * D, (hh + 1) * D) for hh in heads]
            )
            wproj_s = np.ascontiguousarray(
                Wproj[csel, :] * np.tile(subln_w, HPG)[:, None]
            )
            # fold the qk rms affine weights into wk columns
            wk_s = np.ascontiguousarray(
                Wk[:, csel].astype(np.float64) * wqk[None, :]
            ).astype(np.float32)
            bias_tab = np.empty((128, HPG * 16), np.float32)
            vfac_a = np.zeros((128, 2 * 16), np.float32)
            comp_a = np.empty((128, TC * HPG * 512), np.float32)
            ii = np.arange(512, dtype=np.float64)
            for j, hh in enumerate(heads):
                slope = slopes[hh]
                for diff in range(16):
                    bias_tab[:, j * 16 + diff] = (
                        slope * (dj - PIN - 128.0 * diff) + LN_G
                    )
                if j >= 2:
                    for jt in range(16):
                        vfac_a[:, (j - 2) * 16 + jt] = np.float32(
                            math.exp(slope * (128.0 * jt - K_PIN))
                        )
                # per-i factor baked into pt by the pinning scheme:
                #   multi-exp (j<2):  exp(slope*((i mod 128) - PIN))
                #   single-exp (j>=2): exp(slope*(i - PIN - K_PIN)) (global i)
                # comp removes it (G stays in, keeping rows >> eps floor).
                for ic in range(TC):
                    if j < 2:
                        fac = slope * ((ii % 128.0) - PIN)
                    else:
                        fac = slope * (512.0 * ic + ii - PIN - K_PIN)
                    comp_a[:, ic * HPG * 512 + j * 512:
                           ic * HPG * 512 + (j + 1) * 512] = (
                        np.exp(-fac)[None, :].astype(np.float32)
                    )
            in_maps.append({
                "xt": xt,
                "wq": np.ascontiguousarray(Wq[:, csel]),
                "wk": wk_s,
                "wv": np.ascontiguousarray(Wv[:, csel]),
                "wp": wproj_s,
                "bias_tab": bias_tab,
                "cmask": cmask,
                "ones_c": ones_c,
                "ones_r": ones_r,
                "vfac": vfac_a,
                "comp": comp_a,
            })

    return in_maps


def gather(results):
    outs = [r["out"] for r in results]
    final = np.stack(
        [sum(outs[b * HG + 1:(b + 1) * HG], outs[b * HG]) for b in range(B)]
    )
    return final.astype(np.float32)


def kernel(x, Wq, Wk, Wv, Wproj, q_rms_w, k_rms_w, subln_w):
    from concourse.bass_utils import run_bass_kernel_spmd

    in_maps = kernel_in_maps(x, Wq, Wk, Wv, Wproj, q_rms_w, k_rms_w, subln_w)
    res = run_bass_kernel_spmd(_get_nc(), in_maps, core_ids=list(range(8)))
    return gather(res.results)


if __name__ == "__main__":
    rng = np.random.default_rng(0)
    ins = {
        "x": rng.standard_normal((B, T, C), dtype=np.float32),
        "Wq": rng.standard_normal((C, H * D), dtype=np.float32) / math.sqrt(C),
        "Wk": rng.standard_normal((C, H * D), dtype=np.float32) / math.sqrt(C),
        "Wv": rng.standard_normal((C, H * D), dtype=np.float32) / math.sqrt(C),
        "Wproj": rng.standard_normal((H * D, C), dtype=np.float32) * 0.001,
        "q_rms_w": np.ones(D, np.float32),
        "k_rms_w": np.ones(D, np.float32),
        "subln_w": np.ones(D, np.float32),
    }
    y = kernel(**ins)
    print("kernel output", y.shape, y.dtype, float(np.abs(y).mean()))
